# revision 2
# baseline (speedup 1.0000x reference)
"""Trainium2 Bass kernel for nn_DefuzzyLayer: out = x @ rules_outcome.

x: [8192, 4096] f32, rules_outcome: [4096, 4096] f32 -> out: [8192, 4096] f32.

Strategy: data-parallel over batch. Each of the 8 NeuronCores computes a
[1024, 4096] output shard: x_shard @ W with the full W replicated.

Per-core kernel (Tile framework), fp8 DoubleRow path (default):
  - Host quantizes x and (W - 0.5) to fp8 e4m3 (TRN FP8_EXP4 == IEEE e4m3,
    max 240).  Mean-centering W halves its quantization error; the exact
    correction 0.5*rowsum(x) is computed host-side in fp32 and added per
    output row during PSUM eviction (rowsum is 0.01% of the FLOPs).
    Net rel-err ~1.8e-2 (x-quant ~1.3e-2 + w-quant ~1.2e-2 in quadrature).
  - DoubleRow perf mode: both matmul operands are 3D APs [128, 2, free] --
    the PE virtualizes to a 128x256 array, contracting 256 per instruction
    at ~2 fp8 MACs/cell/cycle (~1.5x bf16 throughput measured).
  - Host pre-packs x^T and W into partition-major SBUF layout so every load
    is a fully-contiguous multi-MiB DMA; stores batch one [1024, NB] block
    per DMA on the scalar HWDGE ring, separate from the load ring.
  - x shard stays fully resident in SBUF (32 KiB/partition in fp8);
    W streams through double-buffered n-block tiles.
  - Loop: n-block outer, k-pair middle, m inner; each n-block accumulates 8
    m-tiles into 8 PSUM banks over 16 k-pair steps, then evicts
    PSUM -> (+bias) -> SBUF -> DRAM on the vector engine.
"""

import os

import numpy as np

BATCH = 8192
IN_DIM = 4096
OUT_DIM = 4096
N_CORES = 8
M_SHARD = BATCH // N_CORES  # 1024

P = 128
KT = IN_DIM // P            # 32 k-tiles
MT = M_SHARD // P           # 8 m-tiles

IN_DT = os.environ.get("KDT", "float8e4")  # float8e4 | float16 | bfloat16
NB = int(os.environ.get("KNB", "512" if IN_DT == "float8e4" else "256"))
NBLK = OUT_DIM // NB        # n-blocks
XCHUNKS = int(os.environ.get("KXC", "8"))  # x load split
KPC = KT // XCHUNKS         # k-tiles per x chunk
PS_BUFS = int(os.environ.get("KPSBUFS", "8"))
WBUFS = int(os.environ.get("KWB", "3" if IN_DT == "float8e4" else "3"))

_cached_nc = None


def _np_dt():
    import ml_dtypes
    if IN_DT == "float16":
        return np.dtype(np.float16)
    if IN_DT == "bfloat16":
        return np.dtype(ml_dtypes.bfloat16)
    if IN_DT == "float8e4":
        return np.dtype(ml_dtypes.float8_e4m3)
    return np.dtype(np.float32)


def _build(loop_n=1, in_dt=None, variant="full"):
    """Build + compile the per-core Bass module.

    loop_n > 1 wraps the whole body in an on-device For_i loop -- used only
    for HW timing (amortizes host dispatch overhead out of the measurement).
    variant: "full" | "nodma" (skip x/w loads) | "dmaonly" (skip compute).
    """
    import contextlib
    import concourse.bacc as bacc
    import concourse.tile as tile
    import concourse.mybir as mybir

    do_in_dma = variant not in ("nodma", "mmonly")
    do_compute = variant != "dmaonly"
    do_evict = variant != "mmonly"

    in_dt = in_dt or IN_DT
    fp8 = in_dt == "float8e4"
    dt_in = getattr(mybir.dt, in_dt)

    nc = bacc.Bacc("TRN2", target_bir_lowering=False, debug=False)
    # partition-major packed inputs (see _pack_x_shard/_pack_w)
    xt = nc.dram_tensor(
        "xt", [P, KT, M_SHARD], dt_in, kind="ExternalInput"
    ).ap()
    w = nc.dram_tensor(
        "w", [P, NBLK * KT, NB], dt_in, kind="ExternalInput"
    ).ap()
    if fp8:
        bias = nc.dram_tensor(
            "bias", [P, MT], mybir.dt.float32, kind="ExternalInput"
        ).ap()
    out = nc.dram_tensor(
        "out", [M_SHARD, OUT_DIM], mybir.dt.float32, kind="ExternalOutput"
    ).ap()
    out_r = out.rearrange("(m p) n -> p m n", p=P)  # [128, MT, OUT_DIM]

    with tile.TileContext(nc) as tc:
        loop_ctx = (
            tc.For_i(0, loop_n, 1,
                     hint_engines=(mybir.EngineType.PE, mybir.EngineType.SP,
                                   mybir.EngineType.DVE))
            if loop_n > 1 else contextlib.nullcontext()
        )
        with (
            loop_ctx,
            tc.tile_pool(name="xpool", bufs=XCHUNKS) as xpool,
            tc.tile_pool(name="bpool", bufs=1) as bpool,
            tc.tile_pool(name="wpool", bufs=WBUFS) as wpool,
            tc.tile_pool(name="opool", bufs=2) as opool,
            tc.tile_pool(name="pspool", bufs=PS_BUFS, space="PSUM") as pspool,
        ):
            bias_sb = None
            if fp8:
                bias_sb = bpool.tile([P, MT], mybir.dt.float32,
                                     name="bias", tag="b")
                nc.sync.dma_start(out=bias_sb[:], in_=bias[:, :])
            x_chunks = []
            for c in range(XCHUNKS):
                x_c = xpool.tile([P, KPC, M_SHARD], dt_in,
                                 name=f"x{c}", tag="x")
                if do_in_dma:
                    nc.sync.dma_start(
                        out=x_c[:],
                        in_=xt[:, c * KPC:(c + 1) * KPC, :],
                    )
                else:
                    nc.vector.memset(x_c[:, 0, 0:1], 0.0)
                x_chunks.append(x_c)

            shared_psums = None
            if not do_evict:
                shared_psums = [
                    pspool.tile([P, NB], mybir.dt.float32,
                                name=f"sps{m}", tag="ps")
                    for m in range(MT)
                ]
            for b in range(NBLK):
                w_b = wpool.tile([P, KT, NB], dt_in, name=f"w{b}", tag="w")
                if do_in_dma:
                    nc.sync.dma_start(
                        out=w_b[:],
                        in_=w[:, b * KT:(b + 1) * KT, :],
                    )
                else:
                    nc.vector.memset(w_b[:, 0, 0:1], 0.0)

                if not do_compute:
                    continue
                o_b = None
                if do_evict:
                    o_b = opool.tile([P, MT, NB], mybir.dt.float32,
                                     name=f"o{b}", tag="o")
                if shared_psums is not None:
                    psums = shared_psums
                else:
                    psums = [
                        pspool.tile([P, NB], mybir.dt.float32,
                                    name=f"ps{b}_{m}", tag="ps")
                        for m in range(MT)
                    ]
                if fp8:
                    for kk in range(0, KT, 2):
                        xc = x_chunks[kk // KPC]
                        ko = kk % KPC
                        for m in range(MT):
                            nc.tensor.matmul(
                                psums[m][:],
                                xc[:, ko:ko + 2, m * P:(m + 1) * P],
                                w_b[:, kk:kk + 2, :],
                                start=(kk == 0),
                                stop=(kk == KT - 2),
                                perf_mode=mybir.MatmulPerfMode.DoubleRow,
                            )
                else:
                    for k in range(KT):
                        xc = x_chunks[k // KPC]
                        ko = k % KPC
                        for m in range(MT):
                            nc.tensor.matmul(
                                psums[m][:],
                                xc[:, ko, m * P:(m + 1) * P],
                                w_b[:, k, :],
                                start=(k == 0),
                                stop=(k == KT - 1),
                            )
                if do_evict:
                    for m in range(MT):
                        if fp8:
                            nc.vector.tensor_scalar(
                                o_b[:, m, :], psums[m][:],
                                bias_sb[:, m:m + 1], None,
                                mybir.AluOpType.add,
                            )
                        else:
                            nc.vector.tensor_copy(o_b[:, m, :], psums[m][:])
                    nc.scalar.dma_start(
                        out=out_r[:, :, b * NB:(b + 1) * NB],
                        in_=o_b[:],
                    )

    nc.compile()
    return nc


def _get_nc():
    global _cached_nc
    if _cached_nc is None:
        _cached_nc = _build()
    return _cached_nc


def _pack_x_shard(x_shard_q):
    """[M_SHARD, IN_DIM] -> [128, KT, M_SHARD] partition-major."""
    # dest[p, k, m] = x_shard[m, k*128 + p]
    return np.ascontiguousarray(
        x_shard_q.T.reshape(KT, P, M_SHARD).transpose(1, 0, 2)
    )


def _pack_w(w_q):
    """[IN_DIM, OUT_DIM] -> [128, NBLK*KT, NB] partition-major."""
    # dest[p, b*KT + k, j] = w[k*128 + p, b*NB + j]
    return np.ascontiguousarray(
        w_q.reshape(KT, P, NBLK, NB).transpose(1, 2, 0, 3).reshape(P, NBLK * KT, NB)
    )


def _make_in_maps(x, rules_outcome):
    np_dt = _np_dt()
    fp8 = IN_DT == "float8e4"
    x = np.asarray(x, dtype=np.float32)
    w = np.asarray(rules_outcome, dtype=np.float32)
    assert x.shape == (BATCH, IN_DIM) and w.shape == (IN_DIM, OUT_DIM)
    if fp8:
        w_packed = _pack_w((w - np.float32(0.5)).astype(np_dt))
        # bias[p, mt] = 0.5 * sum_i x[shard + mt*128 + p, i], exact in fp32
        rowsum = 0.5 * x.sum(axis=1, dtype=np.float64).astype(np.float32)
    else:
        w_packed = _pack_w(w.astype(np_dt))
    maps = []
    for i in range(N_CORES):
        m = {
            "xt": _pack_x_shard(
                x[i * M_SHARD:(i + 1) * M_SHARD, :].astype(np_dt)),
            "w": w_packed,
        }
        if fp8:
            m["bias"] = np.ascontiguousarray(
                rowsum[i * M_SHARD:(i + 1) * M_SHARD].reshape(MT, P).T)
        maps.append(m)
    return maps


def _run(x, rules_outcome, **spmd_kwargs):
    from concourse.bass_utils import run_bass_kernel_spmd

    in_maps = _make_in_maps(x, rules_outcome)
    nc = _get_nc()
    res = run_bass_kernel_spmd(nc, in_maps, core_ids=list(range(N_CORES)),
                               **spmd_kwargs)
    full = np.concatenate([res.results[i]["out"] for i in range(N_CORES)],
                          axis=0)
    return full, res


def kernel(x, rules_outcome):
    out, _ = _run(x, rules_outcome)
    return out


# revision 3
# speedup vs baseline: 462.6518x; 462.6518x over previous
"""Trainium2 Bass kernel for nn_DefuzzyLayer: out = x @ rules_outcome.

x: [8192, 4096] f32, rules_outcome: [4096, 4096] f32 -> out: [8192, 4096] f32.

Strategy: data-parallel over batch. Each of the 8 NeuronCores computes a
[1024, 4096] output shard: x_shard @ W with the full W replicated.

Per-core kernel (Tile framework), fp8 DoubleRow path (default):
  - Host quantizes x and (W - 0.5) to fp8 e4m3 (TRN FP8_EXP4 == IEEE e4m3).
    Mean-centering W halves its quantization error; the exact correction
    0.5*rowsum(x) is computed host-side in fp32 and added per output row
    during PSUM eviction.  Net rel-err ~1.8e-2.
  - DoubleRow perf mode: both matmul operands are 3D APs [128, 2, free];
    the PE virtualizes to 128x256, contracting 256 per instruction at
    2 fp8 MACs/cell/cycle (~1.5-2x bf16 throughput).
  - Pipelining: each n-block's 8 m-tiles are processed as two halves of 4
    PSUM banks each, so evictions of one half overlap matmuls of the other
    and PSUM bank reuse never stalls the PE.
  - DMA: w loads on the SP ring (block 0 split into 4 sub-loads so the
    first matmul starts after ~1 MiB arrives), x + bias on the DVE ring,
    stores on the ACT ring.  Eviction converts to fp16 (store traffic
    halves; adds ~5e-4 relative error); host casts back to fp32.
"""

import os

import numpy as np

BATCH = 8192
IN_DIM = 4096
OUT_DIM = 4096
N_CORES = 8
M_SHARD = BATCH // N_CORES  # 1024

P = 128
KT = IN_DIM // P            # 32 k-tiles
MT = M_SHARD // P           # 8 m-tiles

IN_DT = os.environ.get("KDT", "float8e4")  # float8e4 | float16 | bfloat16
NB = int(os.environ.get("KNB", "512" if IN_DT == "float8e4" else "256"))
NBLK = OUT_DIM // NB        # n-blocks
XCHUNKS = int(os.environ.get("KXC", "8"))  # x load split
KPC = KT // XCHUNKS         # k-tiles per x chunk
PS_BUFS = int(os.environ.get("KPSBUFS", "8"))
WBUFS = int(os.environ.get("KWB", "3"))
WSPLIT = int(os.environ.get("KWSPLIT", "4"))   # w block-0 load split
MH = MT // 2                # m-tiles per half
OUT_DT = os.environ.get("KODT", "float16")     # on-device output dtype

_cached_nc = None


def _np_dt():
    import ml_dtypes
    if IN_DT == "float16":
        return np.dtype(np.float16)
    if IN_DT == "bfloat16":
        return np.dtype(ml_dtypes.bfloat16)
    if IN_DT == "float8e4":
        return np.dtype(ml_dtypes.float8_e4m3)
    return np.dtype(np.float32)


def _build(loop_n=1, in_dt=None, variant="full"):
    """Build + compile the per-core Bass module.

    loop_n > 1 wraps the whole body in an on-device For_i loop -- used only
    for HW timing (amortizes host dispatch overhead out of the measurement).
    variant: "full" | "nodma" (skip x/w loads) | "dmaonly" (skip compute)
             | "mmonly" (skip loads + eviction).
    """
    import contextlib
    import concourse.bacc as bacc
    import concourse.tile as tile
    import concourse.mybir as mybir

    do_in_dma = variant not in ("nodma", "mmonly")
    do_compute = variant != "dmaonly"
    do_evict = variant != "mmonly"

    in_dt = in_dt or IN_DT
    fp8 = in_dt == "float8e4"
    dt_in = getattr(mybir.dt, in_dt)
    dt_out = getattr(mybir.dt, OUT_DT)

    nc = bacc.Bacc("TRN2", target_bir_lowering=False, debug=False)
    # partition-major packed inputs (see _pack_x_shard/_pack_w)
    xt = nc.dram_tensor(
        "xt", [P, KT, M_SHARD], dt_in, kind="ExternalInput"
    ).ap()
    w = nc.dram_tensor(
        "w", [P, NBLK * KT, NB], dt_in, kind="ExternalInput"
    ).ap()
    if fp8:
        bias = nc.dram_tensor(
            "bias", [P, MT], mybir.dt.float32, kind="ExternalInput"
        ).ap()
    out = nc.dram_tensor(
        "out", [M_SHARD, OUT_DIM], dt_out, kind="ExternalOutput"
    ).ap()
    out_r = out.rearrange("(m p) n -> p m n", p=P)  # [128, MT, OUT_DIM]

    DR = mybir.MatmulPerfMode.DoubleRow

    with tile.TileContext(nc) as tc:
        loop_ctx = (
            tc.For_i(0, loop_n, 1,
                     hint_engines=(mybir.EngineType.PE, mybir.EngineType.SP,
                                   mybir.EngineType.DVE))
            if loop_n > 1 else contextlib.nullcontext()
        )
        with (
            loop_ctx,
            tc.tile_pool(name="xpool", bufs=XCHUNKS) as xpool,
            tc.tile_pool(name="bpool", bufs=1) as bpool,
            tc.tile_pool(name="w0pool", bufs=WSPLIT) as w0pool,
            tc.tile_pool(name="wpool", bufs=WBUFS) as wpool,
            tc.tile_pool(name="opool", bufs=4) as opool,
            tc.tile_pool(name="pspool", bufs=PS_BUFS, space="PSUM") as pspool,
        ):
            # --- w block 0, split into WSPLIT sub-tiles for fast startup
            KSP = KT // WSPLIT
            w0_tiles = []
            if fp8:
                for q in range(WSPLIT):
                    w0q = w0pool.tile([P, KSP, NB], dt_in,
                                      name=f"w0{q}", tag="w0")
                    if do_in_dma:
                        nc.sync.dma_start(
                            out=w0q[:], in_=w[:, q * KSP:(q + 1) * KSP, :])
                    else:
                        nc.vector.memset(w0q[:, 0, 0:1], 0.0)
                    w0_tiles.append(w0q)

            # --- x shard + bias on the DVE ring
            bias_sb = None
            if fp8:
                bias_sb = bpool.tile([P, MT], mybir.dt.float32,
                                     name="bias", tag="b")
                nc.vector.dma_start(out=bias_sb[:], in_=bias[:, :])
            x_chunks = []
            for c in range(XCHUNKS):
                x_c = xpool.tile([P, KPC, M_SHARD], dt_in,
                                 name=f"x{c}", tag="x")
                if do_in_dma:
                    nc.vector.dma_start(
                        out=x_c[:],
                        in_=xt[:, c * KPC:(c + 1) * KPC, :],
                    )
                else:
                    nc.vector.memset(x_c[:, 0, 0:1], 0.0)
                x_chunks.append(x_c)

            shared_psums = None
            if not do_evict:
                shared_psums = [
                    pspool.tile([P, NB], mybir.dt.float32,
                                name=f"sps{m}", tag="ps")
                    for m in range(MT)
                ]

            def w_slice(b, w_b, kk):
                if fp8 and b == 0:
                    q, r = divmod(kk, KSP)
                    return w0_tiles[q][:, r:r + 2, :]
                return w_b[:, kk:kk + 2, :]

            for b in range(NBLK):
                w_b = None
                if not (fp8 and b == 0):
                    w_b = wpool.tile([P, KT, NB], dt_in, name=f"w{b}", tag="w")
                    if do_in_dma:
                        nc.sync.dma_start(
                            out=w_b[:],
                            in_=w[:, b * KT:(b + 1) * KT, :],
                        )
                    else:
                        nc.vector.memset(w_b[:, 0, 0:1], 0.0)

                if not do_compute:
                    continue
                if fp8:
                    for h in range(2):
                        ms = range(h * MH, (h + 1) * MH)
                        if shared_psums is not None:
                            psums = {m: shared_psums[m] for m in ms}
                        else:
                            psums = {
                                m: pspool.tile([P, NB], mybir.dt.float32,
                                               name=f"ps{b}_{m}", tag="ps")
                                for m in ms
                            }
                        for kk in range(0, KT, 2):
                            xc = x_chunks[kk // KPC]
                            ko = kk % KPC
                            ws = w_slice(b, w_b, kk)
                            for m in ms:
                                nc.tensor.matmul(
                                    psums[m][:],
                                    xc[:, ko:ko + 2, m * P:(m + 1) * P],
                                    ws,
                                    start=(kk == 0),
                                    stop=(kk == KT - 2),
                                    perf_mode=DR,
                                )
                        if do_evict:
                            o_h = opool.tile([P, MH, NB], dt_out,
                                             name=f"o{b}_{h}", tag="o")
                            for i, m in enumerate(ms):
                                nc.vector.tensor_scalar(
                                    o_h[:, i, :], psums[m][:],
                                    bias_sb[:, m:m + 1], None,
                                    mybir.AluOpType.add,
                                )
                            nc.scalar.dma_start(
                                out=out_r[:, h * MH:(h + 1) * MH,
                                          b * NB:(b + 1) * NB],
                                in_=o_h[:],
                            )
                else:
                    o_b = None
                    if do_evict:
                        o_b = opool.tile([P, MT, NB], dt_out,
                                         name=f"o{b}", tag="o")
                    if shared_psums is not None:
                        psums = shared_psums
                    else:
                        psums = [
                            pspool.tile([P, NB], mybir.dt.float32,
                                        name=f"ps{b}_{m}", tag="ps")
                            for m in range(MT)
                        ]
                    for k in range(KT):
                        xc = x_chunks[k // KPC]
                        ko = k % KPC
                        for m in range(MT):
                            nc.tensor.matmul(
                                psums[m][:],
                                xc[:, ko, m * P:(m + 1) * P],
                                w_b[:, k, :],
                                start=(k == 0),
                                stop=(k == KT - 1),
                            )
                    if do_evict:
                        for m in range(MT):
                            nc.vector.tensor_copy(o_b[:, m, :], psums[m][:])
                        nc.scalar.dma_start(
                            out=out_r[:, :, b * NB:(b + 1) * NB],
                            in_=o_b[:],
                        )

    nc.compile()
    return nc


def _get_nc():
    global _cached_nc
    if _cached_nc is None:
        _cached_nc = _build()
    return _cached_nc


def _pack_x_shard(x_shard_q):
    """[M_SHARD, IN_DIM] -> [128, KT, M_SHARD] partition-major."""
    # dest[p, k, m] = x_shard[m, k*128 + p]
    return np.ascontiguousarray(
        x_shard_q.T.reshape(KT, P, M_SHARD).transpose(1, 0, 2)
    )


def _pack_w(w_q):
    """[IN_DIM, OUT_DIM] -> [128, NBLK*KT, NB] partition-major."""
    # dest[p, b*KT + k, j] = w[k*128 + p, b*NB + j]
    return np.ascontiguousarray(
        w_q.reshape(KT, P, NBLK, NB).transpose(1, 2, 0, 3).reshape(P, NBLK * KT, NB)
    )


def _make_in_maps(x, rules_outcome):
    np_dt = _np_dt()
    fp8 = IN_DT == "float8e4"
    x = np.asarray(x, dtype=np.float32)
    w = np.asarray(rules_outcome, dtype=np.float32)
    assert x.shape == (BATCH, IN_DIM) and w.shape == (IN_DIM, OUT_DIM)
    if fp8:
        w_packed = _pack_w((w - np.float32(0.5)).astype(np_dt))
        # bias[p, mt] = 0.5 * sum_i x[shard + mt*128 + p, i], exact in fp32
        rowsum = 0.5 * x.sum(axis=1, dtype=np.float64).astype(np.float32)
    else:
        w_packed = _pack_w(w.astype(np_dt))
    maps = []
    for i in range(N_CORES):
        m = {
            "xt": _pack_x_shard(
                x[i * M_SHARD:(i + 1) * M_SHARD, :].astype(np_dt)),
            "w": w_packed,
        }
        if fp8:
            m["bias"] = np.ascontiguousarray(
                rowsum[i * M_SHARD:(i + 1) * M_SHARD].reshape(MT, P).T)
        maps.append(m)
    return maps


def _run(x, rules_outcome, **spmd_kwargs):
    from concourse.bass_utils import run_bass_kernel_spmd

    in_maps = _make_in_maps(x, rules_outcome)
    nc = _get_nc()
    res = run_bass_kernel_spmd(nc, in_maps, core_ids=list(range(N_CORES)),
                               **spmd_kwargs)
    full = np.concatenate(
        [np.asarray(res.results[i]["out"], dtype=np.float32)
         for i in range(N_CORES)], axis=0)
    return full, res


def kernel(x, rules_outcome):
    out, _ = _run(x, rules_outcome)
    return out
